# revision 9
# baseline (speedup 1.0000x reference)
"""GCN 3-layer message passing kernel for Trainium2 (8 NeuronCores).

Sharding: nodes assigned to cores by degree-rank round-robin; within a core,
dst nodes are sorted lexicographically by their per-src-chunk in-edge counts
(chunk = pair of owner cores) so that each 128-node tile needs near-uniform
slot counts per chunk. Messages are fetched with batched SWDGE dma_gather
(<=1024 int16 indices per instruction) from a replicated table of 64-float
rows rebuilt each layer via AllGather. Self-loops are folded into the
gather schedule. Host does the layer-1 input transform (x@W1*dinv), the
final pooling and the linear head.
"""

import time
import numpy as np

N_NODES = 100000
N_EDGES = 3200000
FEAT = 30
HID = 30
N_GRAPHS = 512
NCORES = 8
NODES_PER_CORE = 12500
P = 128
NTILES = (NODES_PER_CORE + P - 1) // P    # 98
PAD_NODES = NTILES * P                    # 12544
SEG = NODES_PER_CORE + 1                  # 12501 rows/core in table (zero row first)
SEG_PAD = PAD_NODES                       # gown rows padded to 12544 for bulk DMA
NCHUNK = 4                                # cores 2k,2k+1 per chunk; idx < 2*12501+1
ROWW = 64                                 # table row width (fp32) -> 256B rows
GMAX = 1024                               # max idx per dma_gather instruction

_COMPILED = None
_COMPILED_KEY = None


class _Runner:
    """Compile a Bacc kernel once; run it on NCORES cores via PJRT."""

    def __init__(self, nc, n_cores):
        import jax
        import concourse.mybir as mybir
        from concourse.bass2jax import (
            _bass_exec_p, install_neuronx_cc_hook, partition_id_tensor)
        from jax.sharding import Mesh, PartitionSpec
        from jax.experimental.shard_map import shard_map

        install_neuronx_cc_hook()
        self.jax = jax
        self.n_cores = n_cores
        partition_name = (nc.partition_id_tensor.name
                          if nc.partition_id_tensor else None)
        in_names, out_names, out_avals, zero_outs = [], [], [], []
        for alloc in nc.m.functions[0].allocations:
            if not isinstance(alloc, mybir.MemoryLocationSet):
                continue
            name = alloc.memorylocations[0].name
            if alloc.kind == "ExternalInput":
                if name != partition_name:
                    in_names.append(name)
            elif alloc.kind == "ExternalOutput":
                shape = tuple(alloc.tensor_shape)
                dtype = mybir.dt.np(alloc.dtype)
                out_names.append(name)
                out_avals.append(jax.core.ShapedArray(shape, dtype))
                zero_outs.append(np.zeros(shape, dtype))
        self.in_names, self.out_names, self.zero_outs = (
            in_names, out_names, zero_outs)
        n_params, n_outs = len(in_names), len(out_avals)
        all_in_names = in_names + out_names + (
            [partition_name] if partition_name else [])

        def _body(*args):
            operands = list(args)
            if partition_name is not None:
                operands.append(partition_id_tensor())
            return tuple(_bass_exec_p.bind(
                *operands,
                out_avals=tuple(out_avals),
                in_names=tuple(all_in_names),
                out_names=tuple(out_names),
                lowering_input_output_aliases=(),
                sim_require_finite=True,
                sim_require_nnan=True,
                nc=nc,
            ))

        try:
            devices = jax.devices("axon")[:n_cores]
        except RuntimeError:
            devices = jax.devices()[:n_cores]
        mesh = Mesh(np.asarray(devices), ("core",))
        self.fn = jax.jit(
            shard_map(_body, mesh=mesh,
                      in_specs=(PartitionSpec("core"),) * (n_params + n_outs),
                      out_specs=(PartitionSpec("core"),) * n_outs,
                      check_rep=False),
            keep_unused=True,
        )

    def put_inputs(self, in_maps):
        per_core = [[np.asarray(m[name]) for name in self.in_names]
                    for m in in_maps]
        concat_in = [
            np.concatenate([per_core[c][i] for c in range(self.n_cores)],
                           axis=0)
            for i in range(len(self.in_names))
        ]
        self.dev_in = [self.jax.device_put(a) for a in concat_in]
        self.dev_zo = [self.jax.device_put(z) for z in self._zo()]

    def _zo(self):
        return [np.concatenate([z] * self.n_cores, axis=0)
                for z in self.zero_outs]

    def call(self):
        res = self.fn(*self.dev_in, *self.dev_zo)
        self.jax.block_until_ready(res)
        return res

    def burst(self, burst=10):
        self.call()
        t0 = time.time()
        res = None
        for _ in range(burst):
            res = self.fn(*self.dev_in, *self.dev_zo)
        self.jax.block_until_ready(res)
        return (time.time() - t0) / burst

    def results(self, res):
        out = []
        for c in range(self.n_cores):
            d = {}
            for i, name in enumerate(self.out_names):
                full = np.asarray(res[i])
                sz = full.shape[0] // self.n_cores
                d[name] = full[c * sz:(c + 1) * sz]
            out.append(d)
        return out


def _build_schedule(edge_index):
    """Host preprocessing.

    Returns per-core idx streams (int16, wrap-16 layout), per-tile chunk
    widths, the node permutation and dinv.
    """
    src = np.asarray(edge_index[0], dtype=np.int64)
    dst = np.asarray(edge_index[1], dtype=np.int64)

    deg = np.bincount(dst, minlength=N_NODES).astype(np.int64) + 1
    dinv = (1.0 / np.sqrt(np.maximum(deg, 1).astype(np.float64))).astype(
        np.float32)

    # core assignment: round-robin over degree rank (balances edges/core)
    order = np.argsort(-deg, kind="stable")
    core_of_node = np.empty(N_NODES, dtype=np.int64)
    core_of_node[order] = np.arange(N_NODES) % NCORES

    # self-loops folded into the edge list
    loops = np.arange(N_NODES, dtype=np.int64)
    src = np.concatenate([src, loops])
    dst = np.concatenate([dst, loops])

    chunk_of_src = core_of_node[src] // 2          # [E+N]
    dst_core = core_of_node[dst]

    # per-dst chunk counts -> lex sort within each core
    cnt = np.zeros((N_NODES, NCHUNK), dtype=np.int32)
    np.add.at(cnt, (dst, chunk_of_src), 1)

    perm = np.empty(N_NODES, dtype=np.int64)       # new order: perm[newid]=node
    local_of_node = np.empty(N_NODES, dtype=np.int64)
    for c in range(NCORES):
        nodes_c = np.where(core_of_node == c)[0]
        key = np.lexsort((cnt[nodes_c, 3], cnt[nodes_c, 2],
                          cnt[nodes_c, 1], cnt[nodes_c, 0]))
        nodes_sorted = nodes_c[key]
        perm[c * NODES_PER_CORE:(c + 1) * NODES_PER_CORE] = nodes_sorted
        local_of_node[nodes_sorted] = np.arange(NODES_PER_CORE)

    # edge placement keys
    e_core = dst_core
    e_local = local_of_node[dst]
    e_tile = e_local // P
    e_part = e_local % P
    e_chunk = chunk_of_src
    # src table row (within owning core's segment): zero row 0, nodes 1..12500
    src_row_in_seg = 1 + local_of_node[src]
    src_seg = core_of_node[src]
    # idx local to chunk window (window base = segment of core 2k)
    e_idx16 = ((src_seg - 2 * e_chunk) * SEG_PAD
               + src_row_in_seg).astype(np.int64)

    # group = (core, tile, chunk, part); sort edges into groups
    key_order = np.lexsort((e_idx16, e_part, e_chunk, e_tile, e_core))
    g_core = e_core[key_order]
    g_tile = e_tile[key_order]
    g_chunk = e_chunk[key_order]
    g_part = e_part[key_order]
    g_idx = e_idx16[key_order]

    # counts per (core, tile, chunk, part)
    flat = ((g_core * NTILES + g_tile) * NCHUNK + g_chunk) * P + g_part
    nflat = NCORES * NTILES * NCHUNK * P
    counts = np.bincount(flat, minlength=nflat).reshape(
        NCORES, NTILES, NCHUNK, P)
    Dk = counts.max(axis=3)                        # [NCORES, NTILES, NCHUNK]
    # harness cores share one program: use max over cores for widths
    Dk_shared = Dk.max(axis=0)                     # [NTILES, NCHUNK]

    # slot of each edge: within its (c,t,k) group, column = rank within
    # (c,t,k,p) run; offset col base = sum of earlier chunks' widths
    grp_start = np.searchsorted(flat, np.arange(nflat), side="left")
    rank = np.arange(len(flat)) - grp_start[flat]

    col_base_k = np.zeros((NTILES, NCHUNK), dtype=np.int64)
    col_base_k[:, 1:] = np.cumsum(Dk_shared, axis=1)[:, :-1]
    Dtot_t = Dk_shared.sum(axis=1)                 # [NTILES] total cols per tile

    g_col = col_base_k[g_tile, g_chunk] + rank     # column within tile

    # idx arrays per core: value for slot (t, colk, p); padding -> idx 0
    idx_slots = np.zeros((NCORES, NTILES, int(Dk_shared.max()) * NCHUNK, P),
                         dtype=np.int16)
    # (oversized scratch; real width per tile is Dtot_t[t])
    idx_slots[g_core, g_tile, g_col, g_part] = g_idx.astype(np.int16)

    # build gather op list (static across layers): per (t, k): positions
    # cover columns [col_base_k[t,k], +Dk_shared[t,k]) in chops of <=8 cols
    ops = []          # (tile, chunk, msg_col_base, n_idx, idx_col_base16)
    idx_cols16 = 0
    for t in range(NTILES):
        for k in range(NCHUNK):
            w = int(Dk_shared[t, k])
            if w == 0:
                continue
            cb = int(col_base_k[t, k])
            for c0 in range(0, w, GMAX // P):
                g = min(GMAX // P, w - c0)
                n_idx = g * P
                # positions i=0..n-1: p=i%128, col=cb+c0+i//128
                # idx value for position i: idx_slots[core, t, cb+c0+i//128, i%128]
                ops.append((t, k, cb + c0, n_idx, idx_cols16))
                idx_cols16 += n_idx // 16
    # materialize idx streams per core
    idx16 = np.zeros((NCORES, 16, idx_cols16), dtype=np.int16)
    for (t, k, colb, n_idx, icb) in ops:
        for c in range(NCORES):
            blk = idx_slots[c, t, colb:colb + n_idx // P, :]   # [g, P]
            flat_i = blk.reshape(-1)                           # i = col*128+p
            # position i -> (i%16, icb + i//16)
            wrapped = flat_i.reshape(-1, 16).T                 # [16, n/16]
            idx16[c, :, icb:icb + n_idx // 16] = wrapped

    return {
        "perm": perm, "dinv": dinv,
        "Dtot_t": Dtot_t, "ops": ops, "idx_cols16": idx_cols16,
        "idx16": idx16,
    }


def _build_program(Dtot_t, ops, idx_cols16):
    import concourse.bass as bass
    import concourse.bacc as bacc
    import concourse.mybir as mybir
    from concourse.tile import TileContext
    from concourse.masks import make_identity
    from concourse.library_config import mlp

    fp32 = mybir.dt.float32
    i16 = mybir.dt.int16
    nc = bacc.Bacc("TRN2", target_bir_lowering=False, debug=False,
                   num_devices=NCORES)

    g1own = nc.dram_tensor("g1own", [SEG_PAD, ROWW], fp32,
                           kind="ExternalInput").ap()
    idxs_in = nc.dram_tensor("idxs", [P, idx_cols16], i16,
                             kind="ExternalInput").ap()
    dinv_in = nc.dram_tensor("dinv", [NTILES * P, 1], fp32,
                             kind="ExternalInput").ap()
    w2 = nc.dram_tensor("w2", [HID, HID], fp32, kind="ExternalInput").ap()
    w3 = nc.dram_tensor("w3", [HID, HID], fp32, kind="ExternalInput").ap()
    bb = nc.dram_tensor("bb", [P, 3, HID], fp32, kind="ExternalInput").ap()
    h3_out = nc.dram_tensor("h3", [NTILES * P, HID], fp32,
                            kind="ExternalOutput").ap()

    gown = nc.dram_tensor("gown", [SEG_PAD, ROWW], fp32)
    gfull = nc.dram_tensor("gfull", [NCORES * SEG_PAD, ROWW], fp32,
                           addr_space="Shared")

    nfull = NODES_PER_CORE // P          # 97 full tiles
    nrem = NODES_PER_CORE - nfull * P    # 84

    with TileContext(nc) as tc:
        with (
            tc.tile_pool(name="const", bufs=1) as cp,
            tc.tile_pool(name="stage", bufs=1) as stp,
            tc.tile_pool(name="work", bufs=3) as wp,
            tc.tile_pool(name="small", bufs=6) as sp,
            tc.tile_pool(name="psumT", bufs=2, space="PSUM") as ppT,
            tc.tile_pool(name="psumG", bufs=2, space="PSUM") as ppG,
        ):
            nc.gpsimd.load_library(mlp)
            ident = cp.tile([P, P], fp32)
            make_identity(nc, ident[:])
            w2t = cp.tile([HID, HID], fp32)
            nc.sync.dma_start(out=w2t[:], in_=w2[:, :])
            w3t = cp.tile([HID, HID], fp32)
            nc.sync.dma_start(out=w3t[:], in_=w3[:, :])
            bbt = cp.tile([P, 3, HID], fp32)
            nc.sync.dma_start(out=bbt[:], in_=bb[:, :, :])
            dinv_t = cp.tile([P, NTILES], fp32)
            nc.sync.dma_start(
                out=dinv_t[:],
                in_=dinv_in[:, 0].rearrange("(t p) -> p t", p=P),
            )
            idxs = cp.tile([P, idx_cols16], i16)
            nc.sync.dma_start(out=idxs[:], in_=idxs_in[:, :])

            # copy host-provided layer-1 table (incl. zero row/cols) to gown
            ginit = cp.tile([P, NTILES, ROWW], fp32)
            nc.sync.dma_start(
                out=ginit[:],
                in_=g1own[:, :].rearrange("(t p) f -> p t f", p=P),
            )
            nc.sync.dma_start(
                out=gown[:, :].rearrange("(t p) f -> p t f", p=P),
                in_=ginit[:],
            )

            stage = stp.tile([P, NTILES, HID], fp32)

            def publish():
                tc.strict_bb_all_engine_barrier()
                nc.gpsimd.collective_compute(
                    "AllGather", mybir.AluOpType.bypass,
                    replica_groups=[list(range(NCORES))],
                    ins=[gown[:, :]], outs=[gfull[:, :]],
                )
                tc.strict_bb_all_engine_barrier()

            def publish_stage():
                # stage rows (t,p) -> gown rows 1+t*128+p, cols 0:30
                nc.sync.dma_start(
                    out=gown[1:1 + nfull * P, :HID].rearrange(
                        "(t p) f -> p t f", p=P),
                    in_=stage[:, :nfull, :],
                )
                if nrem:
                    nc.sync.dma_start(
                        out=gown[1 + nfull * P:1 + NODES_PER_CORE, :HID],
                        in_=stage[:nrem, nfull, :],
                    )
                publish()

            publish()

            # per-tile ops grouped
            ops_by_tile = [[] for _ in range(NTILES)]
            for (t, k, colb, n_idx, icb) in ops:
                ops_by_tile[t].append((k, colb, n_idx, icb))

            for layer in range(3):
                for t in range(NTILES):
                    D = int(Dtot_t[t])
                    msg = wp.tile([P, D, ROWW], fp32, tag="msg")
                    for (k, colb, n_idx, icb) in ops_by_tile[t]:
                        win = gfull[2 * k * SEG_PAD:(2 * k + 2) * SEG_PAD, :]
                        nc.gpsimd.dma_gather(
                            out_ap=msg[:, colb:colb + n_idx // P, :],
                            in_ap=win,
                            idxs_ap=idxs[:, icb:icb + n_idx // 16],
                            num_idxs=n_idx,
                            num_idxs_reg=n_idx,
                            elem_size=ROWW,
                        )
                    s0 = sp.tile([P, HID], fp32, tag="s0")
                    nc.vector.tensor_reduce(
                        out=s0[:],
                        in_=msg[:, :, :HID].rearrange("p d f -> p f d"),
                        axis=mybir.AxisListType.X, op=mybir.AluOpType.add,
                    )
                    s2 = sp.tile([P, HID], fp32, tag="s2")
                    nc.vector.scalar_tensor_tensor(
                        out=s2[:], in0=s0[:], scalar=dinv_t[:, t:t + 1],
                        in1=bbt[:, layer, :],
                        op0=mybir.AluOpType.mult, op1=mybir.AluOpType.add,
                    )
                    h = sp.tile([P, HID], fp32, tag="h")
                    nc.scalar.activation(
                        h[:], s2[:], mybir.ActivationFunctionType.Relu)
                    if layer < 2:
                        ht_ps = ppT.tile([HID, P], fp32, tag="tps")
                        nc.tensor.transpose(out=ht_ps[:], in_=h[:],
                                            identity=ident[:])
                        ht = sp.tile([HID, P], fp32, tag="ht")
                        nc.vector.tensor_copy(out=ht[:], in_=ht_ps[:])
                        g_ps = ppG.tile([P, HID], fp32, tag="gps")
                        wmat = w2t if layer == 0 else w3t
                        nc.tensor.matmul(out=g_ps[:], lhsT=ht[:], rhs=wmat[:],
                                         start=True, stop=True)
                        nc.vector.tensor_scalar_mul(
                            out=stage[:, t, :], in0=g_ps[:],
                            scalar1=dinv_t[:, t:t + 1])
                    else:
                        nc.vector.tensor_copy(out=stage[:, t, :], in_=h[:])
                if layer < 2:
                    publish_stage()

            nc.sync.dma_start(
                out=h3_out[:, :].rearrange("(t p) f -> p t f", p=P),
                in_=stage[:],
            )

    nc.compile()
    return nc


def kernel(x, edge_index, batch_ids, W1, b1, W2, b2, W3, b3, lin_W, lin_b):
    global _COMPILED, _COMPILED_KEY
    x = np.asarray(x, dtype=np.float32)
    edge_index = np.asarray(edge_index)
    batch_ids = np.asarray(batch_ids)
    W1 = np.asarray(W1, np.float32); b1 = np.asarray(b1, np.float32)
    W2 = np.asarray(W2, np.float32); b2 = np.asarray(b2, np.float32)
    W3 = np.asarray(W3, np.float32); b3 = np.asarray(b3, np.float32)
    lin_W = np.asarray(lin_W, np.float32); lin_b = np.asarray(lin_b, np.float32)

    sched = _build_schedule(edge_index)
    perm, dinv = sched["perm"], sched["dinv"]

    key = (sched["Dtot_t"].tobytes(), tuple(sched["ops"]))
    if _COMPILED is None or _COMPILED_KEY != key:
        nc = _build_program(sched["Dtot_t"], sched["ops"],
                            sched["idx_cols16"])
        _COMPILED = _Runner(nc, NCORES)
        _COMPILED_KEY = key
    r = _COMPILED

    g1 = (x @ W1) * dinv[:, None]
    g1p = g1[perm]          # [100000, 30] in new node order
    dinvp = dinv[perm]

    bbc = np.stack([
        np.broadcast_to(b1, (P, HID)),
        np.broadcast_to(b2, (P, HID)),
        np.broadcast_to(b3, (P, HID)),
    ], axis=1).astype(np.float32)  # [P, 3, HID]

    idx16 = sched["idx16"]          # [NCORES, 16, idx_cols16]
    in_maps = []
    for c in range(NCORES):
        lo = c * NODES_PER_CORE
        g1own = np.zeros((SEG_PAD, ROWW), np.float32)
        g1own[1:1 + NODES_PER_CORE, :FEAT] = g1p[lo:lo + NODES_PER_CORE]
        dv = np.zeros((NTILES * P, 1), np.float32)
        dv[:NODES_PER_CORE, 0] = dinvp[lo:lo + NODES_PER_CORE]
        in_maps.append({
            "g1own": g1own,
            "idxs": np.tile(idx16[c], (8, 1)),
            "dinv": dv,
            "w2": W2, "w3": W3, "bb": bbc,
        })

    r.put_inputs(in_maps)
    res = r.call()
    results = r.results(res)

    h3p = np.concatenate(
        [results[c]["h3"][:NODES_PER_CORE] for c in range(NCORES)], axis=0)
    h3 = np.empty_like(h3p)
    h3[perm] = h3p
    pooled = np.zeros((N_GRAPHS, HID), np.float32)
    np.add.at(pooled, batch_ids.astype(np.int64), h3)
    return pooled @ lin_W + lin_b


# revision 11
# speedup vs baseline: 1.1705x; 1.1705x over previous
"""GCN 3-layer message passing kernel for Trainium2 (8 NeuronCores).

Sharding: nodes assigned to cores by degree-rank round-robin; within a core,
dst nodes are sorted lexicographically by their per-src-chunk in-edge counts
(chunk = pair of owner cores) so that each 128-node tile needs near-uniform
slot counts per chunk. Messages are fetched with batched SWDGE dma_gather
(<=1024 int16 indices per instruction) from a replicated table of 64-float
rows rebuilt each layer via AllGather. Self-loops are folded into the
gather schedule. Host does the layer-1 input transform (x@W1*dinv), the
final pooling and the linear head.
"""

import time
import numpy as np

N_NODES = 100000
N_EDGES = 3200000
FEAT = 30
HID = 30
N_GRAPHS = 512
NCORES = 8
NODES_PER_CORE = 12500
P = 128
NTILES = (NODES_PER_CORE + P - 1) // P    # 98
PAD_NODES = NTILES * P                    # 12544
SEG = NODES_PER_CORE + 1                  # 12501 rows/core in table (zero row first)
SEG_PAD = PAD_NODES                       # gown rows padded to 12544 for bulk DMA
NCHUNK = 4                                # cores 2k,2k+1 per chunk; idx < 2*12501+1
ROWW = 64                                 # table row width (fp32) -> 256B rows
GMAX = 1024                               # max idx per dma_gather instruction

_COMPILED = None
_COMPILED_KEY = None


class _Runner:
    """Compile a Bacc kernel once; run it on NCORES cores via PJRT."""

    def __init__(self, nc, n_cores):
        import jax
        import concourse.mybir as mybir
        from concourse.bass2jax import (
            _bass_exec_p, install_neuronx_cc_hook, partition_id_tensor)
        from jax.sharding import Mesh, PartitionSpec
        from jax.experimental.shard_map import shard_map

        install_neuronx_cc_hook()
        self.jax = jax
        self.n_cores = n_cores
        partition_name = (nc.partition_id_tensor.name
                          if nc.partition_id_tensor else None)
        in_names, out_names, out_avals, zero_outs = [], [], [], []
        for alloc in nc.m.functions[0].allocations:
            if not isinstance(alloc, mybir.MemoryLocationSet):
                continue
            name = alloc.memorylocations[0].name
            if alloc.kind == "ExternalInput":
                if name != partition_name:
                    in_names.append(name)
            elif alloc.kind == "ExternalOutput":
                shape = tuple(alloc.tensor_shape)
                dtype = mybir.dt.np(alloc.dtype)
                out_names.append(name)
                out_avals.append(jax.core.ShapedArray(shape, dtype))
                zero_outs.append(np.zeros(shape, dtype))
        self.in_names, self.out_names, self.zero_outs = (
            in_names, out_names, zero_outs)
        n_params, n_outs = len(in_names), len(out_avals)
        all_in_names = in_names + out_names + (
            [partition_name] if partition_name else [])

        def _body(*args):
            operands = list(args)
            if partition_name is not None:
                operands.append(partition_id_tensor())
            return tuple(_bass_exec_p.bind(
                *operands,
                out_avals=tuple(out_avals),
                in_names=tuple(all_in_names),
                out_names=tuple(out_names),
                lowering_input_output_aliases=(),
                sim_require_finite=True,
                sim_require_nnan=True,
                nc=nc,
            ))

        try:
            devices = jax.devices("axon")[:n_cores]
        except RuntimeError:
            devices = jax.devices()[:n_cores]
        mesh = Mesh(np.asarray(devices), ("core",))
        self.fn = jax.jit(
            shard_map(_body, mesh=mesh,
                      in_specs=(PartitionSpec("core"),) * (n_params + n_outs),
                      out_specs=(PartitionSpec("core"),) * n_outs,
                      check_rep=False),
            keep_unused=True,
        )

    def put_inputs(self, in_maps):
        per_core = [[np.asarray(m[name]) for name in self.in_names]
                    for m in in_maps]
        concat_in = [
            np.concatenate([per_core[c][i] for c in range(self.n_cores)],
                           axis=0)
            for i in range(len(self.in_names))
        ]
        self.dev_in = [self.jax.device_put(a) for a in concat_in]
        self.dev_zo = [self.jax.device_put(z) for z in self._zo()]

    def _zo(self):
        return [np.concatenate([z] * self.n_cores, axis=0)
                for z in self.zero_outs]

    def call(self):
        res = self.fn(*self.dev_in, *self.dev_zo)
        self.jax.block_until_ready(res)
        return res

    def burst(self, burst=10):
        self.call()
        t0 = time.time()
        res = None
        for _ in range(burst):
            res = self.fn(*self.dev_in, *self.dev_zo)
        self.jax.block_until_ready(res)
        return (time.time() - t0) / burst

    def results(self, res):
        out = []
        for c in range(self.n_cores):
            d = {}
            for i, name in enumerate(self.out_names):
                full = np.asarray(res[i])
                sz = full.shape[0] // self.n_cores
                d[name] = full[c * sz:(c + 1) * sz]
            out.append(d)
        return out


def _build_schedule(edge_index):
    """Host preprocessing.

    Returns per-core idx streams (int16, wrap-16 layout), per-tile chunk
    widths, the node permutation and dinv.
    """
    src = np.asarray(edge_index[0], dtype=np.int64)
    dst = np.asarray(edge_index[1], dtype=np.int64)

    deg = np.bincount(dst, minlength=N_NODES).astype(np.int64) + 1
    dinv = (1.0 / np.sqrt(np.maximum(deg, 1).astype(np.float64))).astype(
        np.float32)

    # core assignment: round-robin over degree rank (balances edges/core)
    order = np.argsort(-deg, kind="stable")
    core_of_node = np.empty(N_NODES, dtype=np.int64)
    core_of_node[order] = np.arange(N_NODES) % NCORES

    # self-loops folded into the edge list
    loops = np.arange(N_NODES, dtype=np.int64)
    src = np.concatenate([src, loops])
    dst = np.concatenate([dst, loops])

    chunk_of_src = core_of_node[src] // 2          # [E+N]
    dst_core = core_of_node[dst]

    # per-dst chunk counts -> lex sort within each core
    cnt = np.zeros((N_NODES, NCHUNK), dtype=np.int32)
    np.add.at(cnt, (dst, chunk_of_src), 1)

    perm = np.empty(N_NODES, dtype=np.int64)       # new order: perm[newid]=node
    local_of_node = np.empty(N_NODES, dtype=np.int64)
    for c in range(NCORES):
        nodes_c = np.where(core_of_node == c)[0]
        key = np.lexsort((cnt[nodes_c, 3], cnt[nodes_c, 2],
                          cnt[nodes_c, 1], cnt[nodes_c, 0]))
        nodes_sorted = nodes_c[key]
        perm[c * NODES_PER_CORE:(c + 1) * NODES_PER_CORE] = nodes_sorted
        local_of_node[nodes_sorted] = np.arange(NODES_PER_CORE)

    # edge placement keys
    e_core = dst_core
    e_local = local_of_node[dst]
    e_tile = e_local // P
    e_part = e_local % P
    e_chunk = chunk_of_src
    # src table row (within owning core's segment): zero row 0, nodes 1..12500
    src_row_in_seg = 1 + local_of_node[src]
    src_seg = core_of_node[src]
    # idx local to chunk window (window base = segment of core 2k)
    e_idx16 = ((src_seg - 2 * e_chunk) * SEG_PAD
               + src_row_in_seg).astype(np.int64)

    # group = (core, tile, chunk, part); sort edges into groups
    key_order = np.lexsort((e_idx16, e_part, e_chunk, e_tile, e_core))
    g_core = e_core[key_order]
    g_tile = e_tile[key_order]
    g_chunk = e_chunk[key_order]
    g_part = e_part[key_order]
    g_idx = e_idx16[key_order]

    # counts per (core, tile, chunk, part)
    flat = ((g_core * NTILES + g_tile) * NCHUNK + g_chunk) * P + g_part
    nflat = NCORES * NTILES * NCHUNK * P
    counts = np.bincount(flat, minlength=nflat).reshape(
        NCORES, NTILES, NCHUNK, P)
    Dk = counts.max(axis=3)                        # [NCORES, NTILES, NCHUNK]
    # harness cores share one program: use max over cores for widths
    Dk_shared = Dk.max(axis=0)                     # [NTILES, NCHUNK]

    # slot of each edge: within its (c,t,k) group, column = rank within
    # (c,t,k,p) run; offset col base = sum of earlier chunks' widths
    grp_start = np.searchsorted(flat, np.arange(nflat), side="left")
    rank = np.arange(len(flat)) - grp_start[flat]

    col_base_k = np.zeros((NTILES, NCHUNK), dtype=np.int64)
    col_base_k[:, 1:] = np.cumsum(Dk_shared, axis=1)[:, :-1]
    Dtot_t = Dk_shared.sum(axis=1)                 # [NTILES] total cols per tile

    g_col = col_base_k[g_tile, g_chunk] + rank     # column within tile

    # idx arrays per core: value for slot (t, colk, p); padding -> idx 0
    idx_slots = np.zeros((NCORES, NTILES, int(Dk_shared.max()) * NCHUNK, P),
                         dtype=np.int16)
    # (oversized scratch; real width per tile is Dtot_t[t])
    idx_slots[g_core, g_tile, g_col, g_part] = g_idx.astype(np.int16)

    # build gather op list (static across layers): per (t, k): positions
    # cover columns [col_base_k[t,k], +Dk_shared[t,k]) in chops of <=8 cols
    ops = []          # (tile, chunk, msg_col_base, n_idx, idx_col_base16)
    idx_cols16 = 0
    for t in range(NTILES):
        for k in range(NCHUNK):
            w = int(Dk_shared[t, k])
            if w == 0:
                continue
            cb = int(col_base_k[t, k])
            for c0 in range(0, w, GMAX // P):
                g = min(GMAX // P, w - c0)
                n_idx = g * P
                # positions i=0..n-1: p=i%128, col=cb+c0+i//128
                # idx value for position i: idx_slots[core, t, cb+c0+i//128, i%128]
                ops.append((t, k, cb + c0, n_idx, idx_cols16))
                idx_cols16 += n_idx // 16
    # materialize idx streams per core
    idx16 = np.zeros((NCORES, 16, idx_cols16), dtype=np.int16)
    for (t, k, colb, n_idx, icb) in ops:
        for c in range(NCORES):
            blk = idx_slots[c, t, colb:colb + n_idx // P, :]   # [g, P]
            flat_i = blk.reshape(-1)                           # i = col*128+p
            # position i -> (i%16, icb + i//16)
            wrapped = flat_i.reshape(-1, 16).T                 # [16, n/16]
            idx16[c, :, icb:icb + n_idx // 16] = wrapped

    return {
        "perm": perm, "dinv": dinv,
        "Dtot_t": Dtot_t, "ops": ops, "idx_cols16": idx_cols16,
        "idx16": idx16,
    }


def _build_program(Dtot_t, ops, idx_cols16):
    import concourse.bass as bass
    import concourse.bacc as bacc
    import concourse.mybir as mybir
    from concourse.tile import TileContext
    from concourse.masks import make_identity
    from concourse.library_config import mlp

    fp32 = mybir.dt.float32
    i16 = mybir.dt.int16
    nc = bacc.Bacc("TRN2", target_bir_lowering=False, debug=False,
                   num_devices=NCORES, num_swdge_queues=4)

    g1own = nc.dram_tensor("g1own", [SEG_PAD, ROWW], fp32,
                           kind="ExternalInput").ap()
    idxs_in = nc.dram_tensor("idxs", [P, idx_cols16], i16,
                             kind="ExternalInput").ap()
    dinv_in = nc.dram_tensor("dinv", [NTILES * P, 1], fp32,
                             kind="ExternalInput").ap()
    w2 = nc.dram_tensor("w2", [HID, HID], fp32, kind="ExternalInput").ap()
    w3 = nc.dram_tensor("w3", [HID, HID], fp32, kind="ExternalInput").ap()
    bb = nc.dram_tensor("bb", [P, 3, HID], fp32, kind="ExternalInput").ap()
    h3_out = nc.dram_tensor("h3", [NTILES * P, HID], fp32,
                            kind="ExternalOutput").ap()

    gown = nc.dram_tensor("gown", [SEG_PAD, ROWW], fp32)
    gfull = nc.dram_tensor("gfull", [NCORES * SEG_PAD, ROWW], fp32,
                           addr_space="Shared")

    nfull = NODES_PER_CORE // P          # 97 full tiles
    nrem = NODES_PER_CORE - nfull * P    # 84

    with TileContext(nc) as tc:
        with (
            tc.tile_pool(name="const", bufs=1) as cp,
            tc.tile_pool(name="stage", bufs=1) as stp,
            tc.tile_pool(name="work", bufs=3) as wp,
            tc.tile_pool(name="small", bufs=6) as sp,
            tc.tile_pool(name="psumT", bufs=2, space="PSUM") as ppT,
            tc.tile_pool(name="psumG", bufs=2, space="PSUM") as ppG,
        ):
            nc.gpsimd.load_library(mlp)
            ident = cp.tile([P, P], fp32)
            make_identity(nc, ident[:])
            w2t = cp.tile([HID, HID], fp32)
            nc.sync.dma_start(out=w2t[:], in_=w2[:, :])
            w3t = cp.tile([HID, HID], fp32)
            nc.sync.dma_start(out=w3t[:], in_=w3[:, :])
            bbt = cp.tile([P, 3, HID], fp32)
            nc.sync.dma_start(out=bbt[:], in_=bb[:, :, :])
            dinv_t = cp.tile([P, NTILES], fp32)
            nc.sync.dma_start(
                out=dinv_t[:],
                in_=dinv_in[:, 0].rearrange("(t p) -> p t", p=P),
            )
            idxs = cp.tile([P, idx_cols16], i16)
            nc.sync.dma_start(out=idxs[:], in_=idxs_in[:, :])

            # copy host-provided layer-1 table (incl. zero row/cols) to gown
            ginit = cp.tile([P, NTILES, ROWW], fp32)
            nc.sync.dma_start(
                out=ginit[:],
                in_=g1own[:, :].rearrange("(t p) f -> p t f", p=P),
            )
            nc.sync.dma_start(
                out=gown[:, :].rearrange("(t p) f -> p t f", p=P),
                in_=ginit[:],
            )

            stage = stp.tile([P, NTILES, HID], fp32)

            def publish():
                tc.strict_bb_all_engine_barrier()
                nc.gpsimd.collective_compute(
                    "AllGather", mybir.AluOpType.bypass,
                    replica_groups=[list(range(NCORES))],
                    ins=[gown[:, :]], outs=[gfull[:, :]],
                )
                tc.strict_bb_all_engine_barrier()

            def publish_stage():
                # stage rows (t,p) -> gown rows 1+t*128+p, cols 0:30
                nc.sync.dma_start(
                    out=gown[1:1 + nfull * P, :HID].rearrange(
                        "(t p) f -> p t f", p=P),
                    in_=stage[:, :nfull, :],
                )
                if nrem:
                    nc.sync.dma_start(
                        out=gown[1 + nfull * P:1 + NODES_PER_CORE, :HID],
                        in_=stage[:nrem, nfull, :],
                    )
                publish()

            publish()

            # per-tile ops grouped
            ops_by_tile = [[] for _ in range(NTILES)]
            for (t, k, colb, n_idx, icb) in ops:
                ops_by_tile[t].append((k, colb, n_idx, icb))

            gq = 0
            for layer in range(3):
                for t in range(NTILES):
                    D = int(Dtot_t[t])
                    msg = wp.tile([P, D, ROWW], fp32, tag="msg")
                    for (k, colb, n_idx, icb) in ops_by_tile[t]:
                        win = gfull[2 * k * SEG_PAD:(2 * k + 2) * SEG_PAD, :]
                        nc.gpsimd.dma_gather(
                            out_ap=msg[:, colb:colb + n_idx // P, :],
                            in_ap=win,
                            idxs_ap=idxs[:, icb:icb + n_idx // 16],
                            num_idxs=n_idx,
                            num_idxs_reg=n_idx,
                            elem_size=ROWW,
                            queue_num=gq % 4,
                        )
                        gq += 1
                    s0 = sp.tile([P, HID], fp32, tag="s0")
                    nc.vector.tensor_reduce(
                        out=s0[:],
                        in_=msg[:, :, :HID].rearrange("p d f -> p f d"),
                        axis=mybir.AxisListType.X, op=mybir.AluOpType.add,
                    )
                    s2 = sp.tile([P, HID], fp32, tag="s2")
                    nc.vector.scalar_tensor_tensor(
                        out=s2[:], in0=s0[:], scalar=dinv_t[:, t:t + 1],
                        in1=bbt[:, layer, :],
                        op0=mybir.AluOpType.mult, op1=mybir.AluOpType.add,
                    )
                    h = sp.tile([P, HID], fp32, tag="h")
                    nc.scalar.activation(
                        h[:], s2[:], mybir.ActivationFunctionType.Relu)
                    if layer < 2:
                        ht_ps = ppT.tile([HID, P], fp32, tag="tps")
                        nc.tensor.transpose(out=ht_ps[:], in_=h[:],
                                            identity=ident[:])
                        ht = sp.tile([HID, P], fp32, tag="ht")
                        nc.vector.tensor_copy(out=ht[:], in_=ht_ps[:])
                        g_ps = ppG.tile([P, HID], fp32, tag="gps")
                        wmat = w2t if layer == 0 else w3t
                        nc.tensor.matmul(out=g_ps[:], lhsT=ht[:], rhs=wmat[:],
                                         start=True, stop=True)
                        nc.vector.tensor_scalar_mul(
                            out=stage[:, t, :], in0=g_ps[:],
                            scalar1=dinv_t[:, t:t + 1])
                    else:
                        nc.vector.tensor_copy(out=stage[:, t, :], in_=h[:])
                if layer < 2:
                    publish_stage()

            nc.sync.dma_start(
                out=h3_out[:, :].rearrange("(t p) f -> p t f", p=P),
                in_=stage[:],
            )

    nc.compile()
    return nc


def kernel(x, edge_index, batch_ids, W1, b1, W2, b2, W3, b3, lin_W, lin_b):
    global _COMPILED, _COMPILED_KEY
    x = np.asarray(x, dtype=np.float32)
    edge_index = np.asarray(edge_index)
    batch_ids = np.asarray(batch_ids)
    W1 = np.asarray(W1, np.float32); b1 = np.asarray(b1, np.float32)
    W2 = np.asarray(W2, np.float32); b2 = np.asarray(b2, np.float32)
    W3 = np.asarray(W3, np.float32); b3 = np.asarray(b3, np.float32)
    lin_W = np.asarray(lin_W, np.float32); lin_b = np.asarray(lin_b, np.float32)

    sched = _build_schedule(edge_index)
    perm, dinv = sched["perm"], sched["dinv"]

    key = (sched["Dtot_t"].tobytes(), tuple(sched["ops"]))
    if _COMPILED is None or _COMPILED_KEY != key:
        nc = _build_program(sched["Dtot_t"], sched["ops"],
                            sched["idx_cols16"])
        _COMPILED = _Runner(nc, NCORES)
        _COMPILED_KEY = key
    r = _COMPILED

    g1 = (x @ W1) * dinv[:, None]
    g1p = g1[perm]          # [100000, 30] in new node order
    dinvp = dinv[perm]

    bbc = np.stack([
        np.broadcast_to(b1, (P, HID)),
        np.broadcast_to(b2, (P, HID)),
        np.broadcast_to(b3, (P, HID)),
    ], axis=1).astype(np.float32)  # [P, 3, HID]

    idx16 = sched["idx16"]          # [NCORES, 16, idx_cols16]
    in_maps = []
    for c in range(NCORES):
        lo = c * NODES_PER_CORE
        g1own = np.zeros((SEG_PAD, ROWW), np.float32)
        g1own[1:1 + NODES_PER_CORE, :FEAT] = g1p[lo:lo + NODES_PER_CORE]
        dv = np.zeros((NTILES * P, 1), np.float32)
        dv[:NODES_PER_CORE, 0] = dinvp[lo:lo + NODES_PER_CORE]
        in_maps.append({
            "g1own": g1own,
            "idxs": np.tile(idx16[c], (8, 1)),
            "dinv": dv,
            "w2": W2, "w3": W3, "bb": bbc,
        })

    r.put_inputs(in_maps)
    res = r.call()
    results = r.results(res)

    h3p = np.concatenate(
        [results[c]["h3"][:NODES_PER_CORE] for c in range(NCORES)], axis=0)
    h3 = np.empty_like(h3p)
    h3[perm] = h3p
    pooled = np.zeros((N_GRAPHS, HID), np.float32)
    np.add.at(pooled, batch_ids.astype(np.int64), h3)
    return pooled @ lin_W + lin_b


# revision 18
# speedup vs baseline: 1.1909x; 1.0174x over previous
"""GCN 3-layer message passing kernel for Trainium2 (8 NeuronCores).

Sharding: nodes assigned to cores by degree-rank round-robin; within a core,
dst nodes are sorted lexicographically by their per-src-chunk in-edge counts
(chunk = pair of owner cores) so that each 128-node tile needs near-uniform
slot counts per chunk. Messages are fetched with batched SWDGE dma_gather
(<=1024 int16 indices per instruction) from a replicated table of 64-float
rows rebuilt each layer via AllGather. Self-loops are folded into the
gather schedule. Host does the layer-1 input transform (x@W1*dinv), the
final pooling and the linear head.
"""

import time
import numpy as np

N_NODES = 100000
N_EDGES = 3200000
FEAT = 30
HID = 30
N_GRAPHS = 512
NCORES = 8
NODES_PER_CORE = 12500
P = 128
NTILES = (NODES_PER_CORE + P - 1) // P    # 98
PAD_NODES = NTILES * P                    # 12544
SEG = NODES_PER_CORE + 1                  # 12501 rows/core in table (zero row first)
SEG_PAD = PAD_NODES                       # gown rows padded to 12544 for bulk DMA
NCHUNK = 4                                # cores 2k,2k+1 per chunk; idx < 2*12501+1
ROWW = 64                                 # table row width (fp32) -> 256B rows
GMAX = 1024                               # max idx per dma_gather instruction

_COMPILED = None
_COMPILED_KEY = None

import os as _os
_ABLATE = _os.environ.get("K_ABLATE", "")  # "", "nogather", "nocompute", "nocoll"


class _Runner:
    """Compile a Bacc kernel once; run it on NCORES cores via PJRT."""

    def __init__(self, nc, n_cores):
        import jax
        import concourse.mybir as mybir
        from concourse.bass2jax import (
            _bass_exec_p, install_neuronx_cc_hook, partition_id_tensor)
        from jax.sharding import Mesh, PartitionSpec
        from jax.experimental.shard_map import shard_map

        install_neuronx_cc_hook()
        self.jax = jax
        self.n_cores = n_cores
        partition_name = (nc.partition_id_tensor.name
                          if nc.partition_id_tensor else None)
        in_names, out_names, out_avals, zero_outs = [], [], [], []
        for alloc in nc.m.functions[0].allocations:
            if not isinstance(alloc, mybir.MemoryLocationSet):
                continue
            name = alloc.memorylocations[0].name
            if alloc.kind == "ExternalInput":
                if name != partition_name:
                    in_names.append(name)
            elif alloc.kind == "ExternalOutput":
                shape = tuple(alloc.tensor_shape)
                dtype = mybir.dt.np(alloc.dtype)
                out_names.append(name)
                out_avals.append(jax.core.ShapedArray(shape, dtype))
                zero_outs.append(np.zeros(shape, dtype))
        self.in_names, self.out_names, self.zero_outs = (
            in_names, out_names, zero_outs)
        n_params, n_outs = len(in_names), len(out_avals)
        all_in_names = in_names + out_names + (
            [partition_name] if partition_name else [])

        def _body(*args):
            operands = list(args)
            if partition_name is not None:
                operands.append(partition_id_tensor())
            return tuple(_bass_exec_p.bind(
                *operands,
                out_avals=tuple(out_avals),
                in_names=tuple(all_in_names),
                out_names=tuple(out_names),
                lowering_input_output_aliases=(),
                sim_require_finite=True,
                sim_require_nnan=True,
                nc=nc,
            ))

        try:
            devices = jax.devices("axon")[:n_cores]
        except RuntimeError:
            devices = jax.devices()[:n_cores]
        mesh = Mesh(np.asarray(devices), ("core",))
        self.fn = jax.jit(
            shard_map(_body, mesh=mesh,
                      in_specs=(PartitionSpec("core"),) * (n_params + n_outs),
                      out_specs=(PartitionSpec("core"),) * n_outs,
                      check_rep=False),
            keep_unused=True,
        )

    def put_inputs(self, in_maps):
        per_core = [[np.asarray(m[name]) for name in self.in_names]
                    for m in in_maps]
        concat_in = [
            np.concatenate([per_core[c][i] for c in range(self.n_cores)],
                           axis=0)
            for i in range(len(self.in_names))
        ]
        self.dev_in = [self.jax.device_put(a) for a in concat_in]
        self.dev_zo = [self.jax.device_put(z) for z in self._zo()]

    def _zo(self):
        return [np.concatenate([z] * self.n_cores, axis=0)
                for z in self.zero_outs]

    def call(self):
        res = self.fn(*self.dev_in, *self.dev_zo)
        self.jax.block_until_ready(res)
        return res

    def burst(self, burst=10):
        self.call()
        t0 = time.time()
        res = None
        for _ in range(burst):
            res = self.fn(*self.dev_in, *self.dev_zo)
        self.jax.block_until_ready(res)
        return (time.time() - t0) / burst

    def results(self, res):
        out = []
        for c in range(self.n_cores):
            d = {}
            for i, name in enumerate(self.out_names):
                full = np.asarray(res[i])
                sz = full.shape[0] // self.n_cores
                d[name] = full[c * sz:(c + 1) * sz]
            out.append(d)
        return out


def _build_schedule(edge_index):
    """Host preprocessing.

    Returns per-core idx streams (int16, wrap-16 layout), per-tile chunk
    widths, the node permutation and dinv.
    """
    src = np.asarray(edge_index[0], dtype=np.int64)
    dst = np.asarray(edge_index[1], dtype=np.int64)

    deg = np.bincount(dst, minlength=N_NODES).astype(np.int64) + 1
    dinv = (1.0 / np.sqrt(np.maximum(deg, 1).astype(np.float64))).astype(
        np.float32)

    # core assignment: round-robin over degree rank (balances edges/core)
    order = np.argsort(-deg, kind="stable")
    core_of_node = np.empty(N_NODES, dtype=np.int64)
    core_of_node[order] = np.arange(N_NODES) % NCORES

    # self-loops folded into the edge list
    loops = np.arange(N_NODES, dtype=np.int64)
    src = np.concatenate([src, loops])
    dst = np.concatenate([dst, loops])

    chunk_of_src = core_of_node[src] // 2          # [E+N]
    dst_core = core_of_node[dst]

    # per-dst chunk counts -> lex sort within each core
    cnt = np.zeros((N_NODES, NCHUNK), dtype=np.int32)
    np.add.at(cnt, (dst, chunk_of_src), 1)

    perm = np.empty(N_NODES, dtype=np.int64)       # new order: perm[newid]=node
    local_of_node = np.empty(N_NODES, dtype=np.int64)
    for c in range(NCORES):
        nodes_c = np.where(core_of_node == c)[0]
        key = np.lexsort((cnt[nodes_c, 3], cnt[nodes_c, 2],
                          cnt[nodes_c, 1], cnt[nodes_c, 0]))
        nodes_sorted = nodes_c[key]
        perm[c * NODES_PER_CORE:(c + 1) * NODES_PER_CORE] = nodes_sorted
        local_of_node[nodes_sorted] = np.arange(NODES_PER_CORE)

    # edge placement keys
    e_core = dst_core
    e_local = local_of_node[dst]
    e_tile = e_local // P
    e_part = e_local % P
    e_chunk = chunk_of_src
    # src table row (within owning core's segment): zero row 0, nodes 1..12500
    src_row_in_seg = 1 + local_of_node[src]
    src_seg = core_of_node[src]
    # idx local to chunk window (window base = segment of core 2k)
    e_idx16 = ((src_seg - 2 * e_chunk) * SEG_PAD
               + src_row_in_seg).astype(np.int64)

    # group = (core, tile, chunk, part); sort edges into groups
    key_order = np.lexsort((e_idx16, e_part, e_chunk, e_tile, e_core))
    g_core = e_core[key_order]
    g_tile = e_tile[key_order]
    g_chunk = e_chunk[key_order]
    g_part = e_part[key_order]
    g_idx = e_idx16[key_order]

    # counts per (core, tile, chunk, part)
    flat = ((g_core * NTILES + g_tile) * NCHUNK + g_chunk) * P + g_part
    nflat = NCORES * NTILES * NCHUNK * P
    counts = np.bincount(flat, minlength=nflat).reshape(
        NCORES, NTILES, NCHUNK, P)
    Dk = counts.max(axis=3)                        # [NCORES, NTILES, NCHUNK]
    # harness cores share one program: use max over cores for widths
    Dk_shared = Dk.max(axis=0)                     # [NTILES, NCHUNK]

    # slot of each edge: within its (c,t,k) group, column = rank within
    # (c,t,k,p) run; offset col base = sum of earlier chunks' widths
    grp_start = np.searchsorted(flat, np.arange(nflat), side="left")
    rank = np.arange(len(flat)) - grp_start[flat]

    col_base_k = np.zeros((NTILES, NCHUNK), dtype=np.int64)
    col_base_k[:, 1:] = np.cumsum(Dk_shared, axis=1)[:, :-1]
    Dtot_t = Dk_shared.sum(axis=1)                 # [NTILES] total cols per tile

    g_col = col_base_k[g_tile, g_chunk] + rank     # column within tile

    # idx arrays per core: value for slot (t, colk, p); padding -> idx 0
    idx_slots = np.zeros((NCORES, NTILES, int(Dk_shared.max()) * NCHUNK, P),
                         dtype=np.int16)
    # (oversized scratch; real width per tile is Dtot_t[t])
    idx_slots[g_core, g_tile, g_col, g_part] = g_idx.astype(np.int16)

    # build gather op list (static across layers): per (t, k): positions
    # cover columns [col_base_k[t,k], +Dk_shared[t,k]) in chops of <=8 cols
    ops = []          # (tile, chunk, msg_col_base, n_idx, idx_col_base16)
    idx_cols16 = 0
    for t in range(NTILES):
        for k in range(NCHUNK):
            w = int(Dk_shared[t, k])
            if w == 0:
                continue
            cb = int(col_base_k[t, k])
            for c0 in range(0, w, GMAX // P):
                g = min(GMAX // P, w - c0)
                n_idx = g * P
                # positions i=0..n-1: p=i%128, col=cb+c0+i//128
                # idx value for position i: idx_slots[core, t, cb+c0+i//128, i%128]
                ops.append((t, k, cb + c0, n_idx, idx_cols16))
                idx_cols16 += n_idx // 16
    # materialize idx streams per core
    idx16 = np.zeros((NCORES, 16, idx_cols16), dtype=np.int16)
    for (t, k, colb, n_idx, icb) in ops:
        for c in range(NCORES):
            blk = idx_slots[c, t, colb:colb + n_idx // P, :]   # [g, P]
            flat_i = blk.reshape(-1)                           # i = col*128+p
            # position i -> (i%16, icb + i//16)
            wrapped = flat_i.reshape(-1, 16).T                 # [16, n/16]
            idx16[c, :, icb:icb + n_idx // 16] = wrapped

    return {
        "perm": perm, "dinv": dinv,
        "Dtot_t": Dtot_t, "ops": ops, "idx_cols16": idx_cols16,
        "idx16": idx16,
    }


def _build_program(Dtot_t, ops, idx_cols16):
    import concourse.bass as bass
    import concourse.bacc as bacc
    import concourse.mybir as mybir
    from concourse.tile import TileContext
    from concourse.masks import make_identity
    from concourse.library_config import mlp

    fp32 = mybir.dt.float32
    i16 = mybir.dt.int16
    nc = bacc.Bacc("TRN2", target_bir_lowering=False, debug=False,
                   num_devices=NCORES, num_swdge_queues=4)

    g1own = nc.dram_tensor("g1own", [SEG_PAD, ROWW], fp32,
                           kind="ExternalInput").ap()
    idxs_in = nc.dram_tensor("idxs", [P, idx_cols16], i16,
                             kind="ExternalInput").ap()
    dinv_in = nc.dram_tensor("dinv", [NTILES * P, 1], fp32,
                             kind="ExternalInput").ap()
    w2 = nc.dram_tensor("w2", [HID, HID], fp32, kind="ExternalInput").ap()
    w3 = nc.dram_tensor("w3", [HID, HID], fp32, kind="ExternalInput").ap()
    bb = nc.dram_tensor("bb", [P, 3, HID], fp32, kind="ExternalInput").ap()
    h3_out = nc.dram_tensor("h3", [NTILES * P, HID], fp32,
                            kind="ExternalOutput").ap()

    gown = nc.dram_tensor("gown", [SEG_PAD, ROWW], fp32)
    gfull = nc.dram_tensor("gfull", [NCORES * SEG_PAD, ROWW], fp32,
                           addr_space="Shared")

    nfull = NODES_PER_CORE // P          # 97 full tiles
    nrem = NODES_PER_CORE - nfull * P    # 84

    with TileContext(nc) as tc:
        with (
            tc.tile_pool(name="const", bufs=1) as cp,
            tc.tile_pool(name="stage", bufs=1) as stp,
            tc.tile_pool(name="work", bufs=3) as wp,
            tc.tile_pool(name="small", bufs=6) as sp,
            tc.tile_pool(name="psumT", bufs=2, space="PSUM") as ppT,
            tc.tile_pool(name="psumG", bufs=2, space="PSUM") as ppG,
        ):
            nc.gpsimd.load_library(mlp)
            ident = cp.tile([P, P], fp32)
            make_identity(nc, ident[:])
            w2t = cp.tile([HID, HID], fp32)
            nc.sync.dma_start(out=w2t[:], in_=w2[:, :])
            w3t = cp.tile([HID, HID], fp32)
            nc.sync.dma_start(out=w3t[:], in_=w3[:, :])
            bbt = cp.tile([P, 3, HID], fp32)
            nc.sync.dma_start(out=bbt[:], in_=bb[:, :, :])
            dinv_t = cp.tile([P, NTILES], fp32)
            nc.sync.dma_start(
                out=dinv_t[:],
                in_=dinv_in[:, 0].rearrange("(t p) -> p t", p=P),
            )
            idxs = cp.tile([P, idx_cols16], i16)
            nc.sync.dma_start(out=idxs[:], in_=idxs_in[:, :])

            # copy host-provided layer-1 table (incl. zero row/cols) to gown
            ginit = cp.tile([P, NTILES, ROWW], fp32)
            nc.sync.dma_start(
                out=ginit[:],
                in_=g1own[:, :].rearrange("(t p) f -> p t f", p=P),
            )
            nc.sync.dma_start(
                out=gown[:, :].rearrange("(t p) f -> p t f", p=P),
                in_=ginit[:],
            )

            stage = stp.tile([P, NTILES, HID], fp32)

            def publish():
                if _ABLATE == "nocoll":
                    return
                tc.strict_bb_all_engine_barrier()
                if _ABLATE != "nocc":
                    nc.gpsimd.collective_compute(
                        "AllGather", mybir.AluOpType.bypass,
                        replica_groups=[list(range(NCORES))],
                        ins=[gown[:, :]], outs=[gfull[:, :]],
                    )
                tc.strict_bb_all_engine_barrier()

            def publish_stage():
                # stage rows (t,p) -> gown rows 1+t*128+p, cols 0:30
                nc.sync.dma_start(
                    out=gown[1:1 + nfull * P, :HID].rearrange(
                        "(t p) f -> p t f", p=P),
                    in_=stage[:, :nfull, :],
                )
                if nrem:
                    nc.sync.dma_start(
                        out=gown[1 + nfull * P:1 + NODES_PER_CORE, :HID],
                        in_=stage[:nrem, nfull, :],
                    )
                publish()

            publish()

            # per-tile ops grouped
            ops_by_tile = [[] for _ in range(NTILES)]
            for (t, k, colb, n_idx, icb) in ops:
                ops_by_tile[t].append((k, colb, n_idx, icb))

            gq = 0
            for layer in range(3):
                for t in range(NTILES):
                    D = int(Dtot_t[t])
                    msg = wp.tile([P, D, ROWW], fp32, tag="msg")
                    if _ABLATE != "nogather":
                        for (k, colb, n_idx, icb) in ops_by_tile[t]:
                            win = gfull[2 * k * SEG_PAD:
                                        (2 * k + 2) * SEG_PAD, :]
                            nc.gpsimd.dma_gather(
                                out_ap=msg[:, colb:colb + n_idx // P, :],
                                in_ap=win,
                                idxs_ap=idxs[:, icb:icb + n_idx // 16],
                                num_idxs=n_idx,
                                num_idxs_reg=n_idx,
                                elem_size=ROWW,
                                queue_num=0,
                            )
                            gq += 1
                    if _ABLATE == "nocompute":
                        continue
                    s0 = sp.tile([P, HID], fp32, tag="s0")
                    if _ABLATE == "noreduce":
                        nc.vector.memset(s0[:], 0.0)
                    else:
                        nc.vector.tensor_reduce(
                            out=s0[:],
                            in_=msg[:, :, :HID].rearrange("p d f -> p f d"),
                            axis=mybir.AxisListType.X, op=mybir.AluOpType.add,
                        )
                    s2 = sp.tile([P, HID], fp32, tag="s2")
                    nc.vector.scalar_tensor_tensor(
                        out=s2[:], in0=s0[:], scalar=dinv_t[:, t:t + 1],
                        in1=bbt[:, layer, :],
                        op0=mybir.AluOpType.mult, op1=mybir.AluOpType.add,
                    )
                    h = sp.tile([P, HID], fp32, tag="h")
                    nc.scalar.activation(
                        h[:], s2[:], mybir.ActivationFunctionType.Relu)
                    if layer < 2:
                        ht_ps = ppT.tile([HID, P], fp32, tag="tps")
                        nc.tensor.transpose(out=ht_ps[:], in_=h[:],
                                            identity=ident[:])
                        ht = sp.tile([HID, P], fp32, tag="ht")
                        nc.vector.tensor_copy(out=ht[:], in_=ht_ps[:])
                        g_ps = ppG.tile([P, HID], fp32, tag="gps")
                        wmat = w2t if layer == 0 else w3t
                        nc.tensor.matmul(out=g_ps[:], lhsT=ht[:], rhs=wmat[:],
                                         start=True, stop=True)
                        nc.vector.tensor_scalar_mul(
                            out=stage[:, t, :], in0=g_ps[:],
                            scalar1=dinv_t[:, t:t + 1])
                    else:
                        nc.vector.tensor_copy(out=stage[:, t, :], in_=h[:])
                if layer < 2:
                    publish_stage()

            nc.sync.dma_start(
                out=h3_out[:, :].rearrange("(t p) f -> p t f", p=P),
                in_=stage[:],
            )

    # SWDGE queue must be consistent with the Tile-assigned DMASW sem lane
    # (each DMASW sem is locked to one queue by the ucode): queue = lane % 4.
    for g in nc.all_instructions():
        if isinstance(g, mybir.InstDMAGatherAnt):
            ups = g.sync_info.on_update if g.sync_info else None
            if ups:
                name = ups[0].ant_name          # e.g. "DMASW3_49"
                lane = int(name[5:name.index("_")])
                g.queue_num = lane % 4

    nc.compile()
    return nc


def kernel(x, edge_index, batch_ids, W1, b1, W2, b2, W3, b3, lin_W, lin_b):
    global _COMPILED, _COMPILED_KEY
    x = np.asarray(x, dtype=np.float32)
    edge_index = np.asarray(edge_index)
    batch_ids = np.asarray(batch_ids)
    W1 = np.asarray(W1, np.float32); b1 = np.asarray(b1, np.float32)
    W2 = np.asarray(W2, np.float32); b2 = np.asarray(b2, np.float32)
    W3 = np.asarray(W3, np.float32); b3 = np.asarray(b3, np.float32)
    lin_W = np.asarray(lin_W, np.float32); lin_b = np.asarray(lin_b, np.float32)

    sched = _build_schedule(edge_index)
    perm, dinv = sched["perm"], sched["dinv"]

    key = (sched["Dtot_t"].tobytes(), tuple(sched["ops"]))
    if _COMPILED is None or _COMPILED_KEY != key:
        nc = _build_program(sched["Dtot_t"], sched["ops"],
                            sched["idx_cols16"])
        _COMPILED = _Runner(nc, NCORES)
        _COMPILED_KEY = key
    r = _COMPILED

    g1 = (x @ W1) * dinv[:, None]
    g1p = g1[perm]          # [100000, 30] in new node order
    dinvp = dinv[perm]

    bbc = np.stack([
        np.broadcast_to(b1, (P, HID)),
        np.broadcast_to(b2, (P, HID)),
        np.broadcast_to(b3, (P, HID)),
    ], axis=1).astype(np.float32)  # [P, 3, HID]

    idx16 = sched["idx16"]          # [NCORES, 16, idx_cols16]
    in_maps = []
    for c in range(NCORES):
        lo = c * NODES_PER_CORE
        g1own = np.zeros((SEG_PAD, ROWW), np.float32)
        g1own[1:1 + NODES_PER_CORE, :FEAT] = g1p[lo:lo + NODES_PER_CORE]
        dv = np.zeros((NTILES * P, 1), np.float32)
        dv[:NODES_PER_CORE, 0] = dinvp[lo:lo + NODES_PER_CORE]
        in_maps.append({
            "g1own": g1own,
            "idxs": np.tile(idx16[c], (8, 1)),
            "dinv": dv,
            "w2": W2, "w3": W3, "bb": bbc,
        })

    r.put_inputs(in_maps)
    res = r.call()
    results = r.results(res)

    h3p = np.concatenate(
        [results[c]["h3"][:NODES_PER_CORE] for c in range(NCORES)], axis=0)
    h3 = np.empty_like(h3p)
    h3[perm] = h3p
    pooled = np.zeros((N_GRAPHS, HID), np.float32)
    np.add.at(pooled, batch_ids.astype(np.int64), h3)
    return pooled @ lin_W + lin_b


# revision 24
# speedup vs baseline: 1.4771x; 1.2403x over previous
"""GCN 3-layer message passing kernel for Trainium2 (8 NeuronCores).

Sharding: nodes assigned to cores by degree-rank round-robin; within a core,
dst nodes are sorted lexicographically by their per-src-chunk in-edge counts
(chunk = pair of owner cores) so that each 128-node tile needs near-uniform
slot counts per chunk. Messages are fetched with batched SWDGE dma_gather
(<=1024 int16 indices per instruction) from a replicated table of 64-float
rows rebuilt each layer via AllGather. Self-loops are folded into the
gather schedule. Host does the layer-1 input transform (x@W1*dinv), the
final pooling and the linear head.
"""

import time
import numpy as np

N_NODES = 100000
N_EDGES = 3200000
FEAT = 30
HID = 30
N_GRAPHS = 512
NCORES = 8
NODES_PER_CORE = 12500
P = 128
NTILES = (NODES_PER_CORE + P - 1) // P    # 98
PAD_NODES = NTILES * P                    # 12544
SEG = NODES_PER_CORE + 1                  # 12501 rows/core in table (zero row first)
SEG_PAD = PAD_NODES                       # gown rows padded to 12544 for bulk DMA
NCHUNK = 4                                # cores 2k,2k+1 per chunk; idx < 2*12501+1
ROWW = 64                                 # table row width (fp32) -> 256B rows
GMAX = 1024                               # max idx per dma_gather instruction

_COMPILED = None
_COMPILED_KEY = None

import os as _os
_ABLATE = _os.environ.get("K_ABLATE", "")  # "", "nogather", "nocompute", "nocoll"


class _Runner:
    """Compile a Bacc kernel once; run it on NCORES cores via PJRT."""

    def __init__(self, nc, n_cores):
        import jax
        import concourse.mybir as mybir
        from concourse.bass2jax import (
            _bass_exec_p, install_neuronx_cc_hook, partition_id_tensor)
        from jax.sharding import Mesh, PartitionSpec
        from jax.experimental.shard_map import shard_map

        install_neuronx_cc_hook()
        self.jax = jax
        self.n_cores = n_cores
        partition_name = (nc.partition_id_tensor.name
                          if nc.partition_id_tensor else None)
        in_names, out_names, out_avals, zero_outs = [], [], [], []
        for alloc in nc.m.functions[0].allocations:
            if not isinstance(alloc, mybir.MemoryLocationSet):
                continue
            name = alloc.memorylocations[0].name
            if alloc.kind == "ExternalInput":
                if name != partition_name:
                    in_names.append(name)
            elif alloc.kind == "ExternalOutput":
                shape = tuple(alloc.tensor_shape)
                dtype = mybir.dt.np(alloc.dtype)
                out_names.append(name)
                out_avals.append(jax.core.ShapedArray(shape, dtype))
                zero_outs.append(np.zeros(shape, dtype))
        self.in_names, self.out_names, self.zero_outs = (
            in_names, out_names, zero_outs)
        n_params, n_outs = len(in_names), len(out_avals)
        all_in_names = in_names + out_names + (
            [partition_name] if partition_name else [])

        def _body(*args):
            operands = list(args)
            if partition_name is not None:
                operands.append(partition_id_tensor())
            return tuple(_bass_exec_p.bind(
                *operands,
                out_avals=tuple(out_avals),
                in_names=tuple(all_in_names),
                out_names=tuple(out_names),
                lowering_input_output_aliases=(),
                sim_require_finite=True,
                sim_require_nnan=True,
                nc=nc,
            ))

        try:
            devices = jax.devices("axon")[:n_cores]
        except RuntimeError:
            devices = jax.devices()[:n_cores]
        mesh = Mesh(np.asarray(devices), ("core",))
        self.fn = jax.jit(
            shard_map(_body, mesh=mesh,
                      in_specs=(PartitionSpec("core"),) * (n_params + n_outs),
                      out_specs=(PartitionSpec("core"),) * n_outs,
                      check_rep=False),
            keep_unused=True,
        )

    def put_inputs(self, in_maps):
        per_core = [[np.asarray(m[name]) for name in self.in_names]
                    for m in in_maps]
        concat_in = [
            np.concatenate([per_core[c][i] for c in range(self.n_cores)],
                           axis=0)
            for i in range(len(self.in_names))
        ]
        self.dev_in = [self.jax.device_put(a) for a in concat_in]
        self.dev_zo = [self.jax.device_put(z) for z in self._zo()]

    def _zo(self):
        return [np.concatenate([z] * self.n_cores, axis=0)
                for z in self.zero_outs]

    def call(self):
        res = self.fn(*self.dev_in, *self.dev_zo)
        self.jax.block_until_ready(res)
        return res

    def burst(self, burst=10):
        self.call()
        t0 = time.time()
        res = None
        for _ in range(burst):
            res = self.fn(*self.dev_in, *self.dev_zo)
        self.jax.block_until_ready(res)
        return (time.time() - t0) / burst

    def results(self, res):
        out = []
        for c in range(self.n_cores):
            d = {}
            for i, name in enumerate(self.out_names):
                full = np.asarray(res[i])
                sz = full.shape[0] // self.n_cores
                d[name] = full[c * sz:(c + 1) * sz]
            out.append(d)
        return out


def _build_schedule(edge_index):
    """Host preprocessing.

    Returns per-core idx streams (int16, wrap-16 layout), per-tile chunk
    widths, the node permutation and dinv.
    """
    src = np.asarray(edge_index[0], dtype=np.int64)
    dst = np.asarray(edge_index[1], dtype=np.int64)

    deg = np.bincount(dst, minlength=N_NODES).astype(np.int64) + 1
    dinv = (1.0 / np.sqrt(np.maximum(deg, 1).astype(np.float64))).astype(
        np.float32)

    # core assignment: round-robin over degree rank (balances edges/core)
    order = np.argsort(-deg, kind="stable")
    core_of_node = np.empty(N_NODES, dtype=np.int64)
    core_of_node[order] = np.arange(N_NODES) % NCORES

    # self-loops folded into the edge list
    loops = np.arange(N_NODES, dtype=np.int64)
    src = np.concatenate([src, loops])
    dst = np.concatenate([dst, loops])

    chunk_of_src = core_of_node[src] // 2          # [E+N]
    dst_core = core_of_node[dst]

    # per-dst chunk counts -> lex sort within each core
    cnt = np.zeros((N_NODES, NCHUNK), dtype=np.int32)
    np.add.at(cnt, (dst, chunk_of_src), 1)

    perm = np.empty(N_NODES, dtype=np.int64)       # new order: perm[newid]=node
    local_of_node = np.empty(N_NODES, dtype=np.int64)
    for c in range(NCORES):
        nodes_c = np.where(core_of_node == c)[0]
        key = np.lexsort((cnt[nodes_c, 3], cnt[nodes_c, 2],
                          cnt[nodes_c, 1], cnt[nodes_c, 0]))
        nodes_sorted = nodes_c[key]
        perm[c * NODES_PER_CORE:(c + 1) * NODES_PER_CORE] = nodes_sorted
        local_of_node[nodes_sorted] = np.arange(NODES_PER_CORE)

    # edge placement keys
    e_core = dst_core
    e_local = local_of_node[dst]
    e_tile = e_local // P
    e_part = e_local % P
    e_chunk = chunk_of_src
    # src table row (within owning core's segment): zero row 0, nodes 1..12500
    src_row_in_seg = 1 + local_of_node[src]
    src_seg = core_of_node[src]
    # idx local to chunk window (window base = segment of core 2k)
    e_idx16 = ((src_seg - 2 * e_chunk) * SEG_PAD
               + src_row_in_seg).astype(np.int64)

    # group = (core, tile, chunk, part); sort edges into groups
    key_order = np.lexsort((e_idx16, e_part, e_chunk, e_tile, e_core))
    g_core = e_core[key_order]
    g_tile = e_tile[key_order]
    g_chunk = e_chunk[key_order]
    g_part = e_part[key_order]
    g_idx = e_idx16[key_order]

    # counts per (core, tile, chunk, part)
    flat = ((g_core * NTILES + g_tile) * NCHUNK + g_chunk) * P + g_part
    nflat = NCORES * NTILES * NCHUNK * P
    counts = np.bincount(flat, minlength=nflat).reshape(
        NCORES, NTILES, NCHUNK, P)
    Dk = counts.max(axis=3)                        # [NCORES, NTILES, NCHUNK]
    # harness cores share one program: use max over cores for widths
    Dk_shared = Dk.max(axis=0)                     # [NTILES, NCHUNK]

    # slot of each edge: within its (c,t,k) group, column = rank within
    # (c,t,k,p) run; offset col base = sum of earlier chunks' widths
    grp_start = np.searchsorted(flat, np.arange(nflat), side="left")
    rank = np.arange(len(flat)) - grp_start[flat]

    col_base_k = np.zeros((NTILES, NCHUNK), dtype=np.int64)
    col_base_k[:, 1:] = np.cumsum(Dk_shared, axis=1)[:, :-1]
    Dtot_t = Dk_shared.sum(axis=1)                 # [NTILES] total cols per tile

    g_col = col_base_k[g_tile, g_chunk] + rank     # column within tile

    # idx arrays per core: value for slot (t, colk, p); padding -> idx 0
    idx_slots = np.zeros((NCORES, NTILES, int(Dk_shared.max()) * NCHUNK, P),
                         dtype=np.int16)
    # (oversized scratch; real width per tile is Dtot_t[t])
    idx_slots[g_core, g_tile, g_col, g_part] = g_idx.astype(np.int16)

    # build gather op list (static across layers): per (t, k): positions
    # cover columns [col_base_k[t,k], +Dk_shared[t,k]) in chops of <=8 cols
    ops = []          # (tile, chunk, msg_col_base, n_idx, idx_col_base16)
    idx_cols16 = 0
    for t in range(NTILES):
        for k in range(NCHUNK):
            w = int(Dk_shared[t, k])
            if w == 0:
                continue
            cb = int(col_base_k[t, k])
            for c0 in range(0, w, GMAX // P):
                g = min(GMAX // P, w - c0)
                n_idx = g * P
                # positions i=0..n-1: p=i%128, col=cb+c0+i//128
                # idx value for position i: idx_slots[core, t, cb+c0+i//128, i%128]
                ops.append((t, k, cb + c0, n_idx, idx_cols16))
                idx_cols16 += n_idx // 16
    # materialize idx streams per core
    idx16 = np.zeros((NCORES, 16, idx_cols16), dtype=np.int16)
    for (t, k, colb, n_idx, icb) in ops:
        for c in range(NCORES):
            blk = idx_slots[c, t, colb:colb + n_idx // P, :]   # [g, P]
            flat_i = blk.reshape(-1)                           # i = col*128+p
            # position i -> (i%16, icb + i//16)
            wrapped = flat_i.reshape(-1, 16).T                 # [16, n/16]
            idx16[c, :, icb:icb + n_idx // 16] = wrapped

    return {
        "perm": perm, "dinv": dinv,
        "Dtot_t": Dtot_t, "ops": ops, "idx_cols16": idx_cols16,
        "idx16": idx16,
    }


def _build_program(Dtot_t, ops, idx_cols16):
    import concourse.bass as bass
    import concourse.bacc as bacc
    import concourse.mybir as mybir
    from concourse.tile import TileContext
    from concourse.masks import make_identity
    from concourse.library_config import mlp

    fp32 = mybir.dt.float32
    i16 = mybir.dt.int16
    nc = bacc.Bacc("TRN2", target_bir_lowering=False, debug=False,
                   num_devices=NCORES, num_swdge_queues=4)

    g1own = nc.dram_tensor("g1own", [SEG_PAD, ROWW], fp32,
                           kind="ExternalInput").ap()
    idxs_in = nc.dram_tensor("idxs", [P, idx_cols16], i16,
                             kind="ExternalInput").ap()
    dinv_in = nc.dram_tensor("dinv", [NTILES * P, 1], fp32,
                             kind="ExternalInput").ap()
    w2 = nc.dram_tensor("w2", [HID, HID], fp32, kind="ExternalInput").ap()
    w3 = nc.dram_tensor("w3", [HID, HID], fp32, kind="ExternalInput").ap()
    bb = nc.dram_tensor("bb", [P, 3, HID], fp32, kind="ExternalInput").ap()
    h3_out = nc.dram_tensor("h3", [NTILES * P, HID], fp32,
                            kind="ExternalOutput").ap()

    gown = nc.dram_tensor("gown", [SEG_PAD, ROWW], fp32)
    gfull = nc.dram_tensor("gfull", [NCORES * SEG_PAD, ROWW], fp32,
                           addr_space="Shared")

    nfull = NODES_PER_CORE // P          # 97 full tiles
    nrem = NODES_PER_CORE - nfull * P    # 84

    with TileContext(nc) as tc:
        with (
            tc.tile_pool(name="const", bufs=1) as cp,
            tc.tile_pool(name="stage", bufs=1) as stp,
            tc.tile_pool(name="work", bufs=3) as wp,
            tc.tile_pool(name="small", bufs=6) as sp,
            tc.tile_pool(name="psumT", bufs=2, space="PSUM") as ppT,
            tc.tile_pool(name="psumG", bufs=2, space="PSUM") as ppG,
        ):
            nc.gpsimd.load_library(mlp)
            ident = cp.tile([P, P], fp32)
            make_identity(nc, ident[:])
            w2t = cp.tile([HID, HID], fp32)
            nc.sync.dma_start(out=w2t[:], in_=w2[:, :])
            w3t = cp.tile([HID, HID], fp32)
            nc.sync.dma_start(out=w3t[:], in_=w3[:, :])
            bbt = cp.tile([P, 3, HID], fp32)
            nc.sync.dma_start(out=bbt[:], in_=bb[:, :, :])
            dinv_t = cp.tile([P, NTILES], fp32)
            nc.sync.dma_start(
                out=dinv_t[:],
                in_=dinv_in[:, 0].rearrange("(t p) -> p t", p=P),
            )
            idxs = cp.tile([P, idx_cols16], i16)
            nc.sync.dma_start(out=idxs[:], in_=idxs_in[:, :])

            # copy host-provided layer-1 table (incl. zero row/cols) to gown
            ginit = cp.tile([P, NTILES, ROWW], fp32)
            nc.sync.dma_start(
                out=ginit[:],
                in_=g1own[:, :].rearrange("(t p) f -> p t f", p=P),
            )
            nc.sync.dma_start(
                out=gown[:, :].rearrange("(t p) f -> p t f", p=P),
                in_=ginit[:],
            )

            stage = stp.tile([P, NTILES, HID], fp32)
            if _ABLATE in ("gonly", "gonly_nc", "nocompute", "gpure"):
                nc.vector.memset(stage[:], 0.0)

            def publish():
                if _ABLATE in ("nocoll", "gonly_nc", "gpure"):
                    return
                tc.strict_bb_all_engine_barrier()
                if _ABLATE != "nocc":
                    nc.gpsimd.collective_compute(
                        "AllGather", mybir.AluOpType.bypass,
                        replica_groups=[list(range(NCORES))],
                        ins=[gown[:, :]], outs=[gfull[:, :]],
                    )
                tc.strict_bb_all_engine_barrier()

            def publish_stage():
                # stage rows (t,p) -> gown rows 1+t*128+p, cols 0:30
                nc.sync.dma_start(
                    out=gown[1:1 + nfull * P, :HID].rearrange(
                        "(t p) f -> p t f", p=P),
                    in_=stage[:, :nfull, :],
                )
                if nrem:
                    nc.sync.dma_start(
                        out=gown[1 + nfull * P:1 + NODES_PER_CORE, :HID],
                        in_=stage[:nrem, nfull, :],
                    )
                publish()

            publish()

            # per-tile ops grouped
            ops_by_tile = [[] for _ in range(NTILES)]
            for (t, k, colb, n_idx, icb) in ops:
                ops_by_tile[t].append((k, colb, n_idx, icb))

            gq = 0
            for layer in range(3):
                for t in range(NTILES):
                    D = int(Dtot_t[t])
                    msg = wp.tile([P, D, ROWW], fp32, tag="msg")
                    if _ABLATE != "nogather":
                        for (k, colb, n_idx, icb) in ops_by_tile[t]:
                            win = gfull[2 * k * SEG_PAD:
                                        (2 * k + 2) * SEG_PAD, :]
                            nc.gpsimd.dma_gather(
                                out_ap=msg[:, colb:colb + n_idx // P, :],
                                in_ap=win,
                                idxs_ap=idxs[:, icb:icb + n_idx // 16],
                                num_idxs=n_idx,
                                num_idxs_reg=n_idx,
                                elem_size=ROWW,
                                queue_num=0,
                            )
                            gq += 1
                    if _ABLATE == "gpure":
                        continue
                    if _ABLATE in ("gonly", "gonly_nc"):
                        tiny = sp.tile([P, 8], fp32, tag="tiny")
                        nc.vector.tensor_copy(out=tiny[:], in_=msg[:, 0, :8])
                        continue
                    if _ABLATE == "nocompute":
                        continue
                    s0 = sp.tile([P, HID], fp32, tag="s0")
                    if _ABLATE == "noreduce":
                        nc.vector.memset(s0[:], 0.0)
                    else:
                        nc.vector.tensor_reduce(
                            out=s0[:],
                            in_=msg[:, :, :HID].rearrange("p d f -> p f d"),
                            axis=mybir.AxisListType.X, op=mybir.AluOpType.add,
                        )
                    s2 = sp.tile([P, HID], fp32, tag="s2")
                    nc.vector.scalar_tensor_tensor(
                        out=s2[:], in0=s0[:], scalar=dinv_t[:, t:t + 1],
                        in1=bbt[:, layer, :],
                        op0=mybir.AluOpType.mult, op1=mybir.AluOpType.add,
                    )
                    h = sp.tile([P, HID], fp32, tag="h")
                    nc.scalar.activation(
                        h[:], s2[:], mybir.ActivationFunctionType.Relu)
                    if layer < 2:
                        ht_ps = ppT.tile([HID, P], fp32, tag="tps")
                        nc.tensor.transpose(out=ht_ps[:], in_=h[:],
                                            identity=ident[:])
                        ht = sp.tile([HID, P], fp32, tag="ht")
                        nc.vector.tensor_copy(out=ht[:], in_=ht_ps[:])
                        g_ps = ppG.tile([P, HID], fp32, tag="gps")
                        wmat = w2t if layer == 0 else w3t
                        nc.tensor.matmul(out=g_ps[:], lhsT=ht[:], rhs=wmat[:],
                                         start=True, stop=True)
                        nc.vector.tensor_scalar_mul(
                            out=stage[:, t, :], in0=g_ps[:],
                            scalar1=dinv_t[:, t:t + 1])
                    else:
                        nc.vector.tensor_copy(out=stage[:, t, :], in_=h[:])
                if layer < 2:
                    publish_stage()

            nc.sync.dma_start(
                out=h3_out[:, :].rearrange("(t p) f -> p t f", p=P),
                in_=stage[:],
            )

    # SWDGE queue must be consistent with the Tile-assigned DMASW sem lane
    # (each DMASW sem is locked to one queue by the ucode): queue = lane % 4.
    for g in nc.all_instructions():
        if isinstance(g, mybir.InstDMAGatherAnt):
            ups = g.sync_info.on_update if g.sync_info else None
            if ups:
                name = ups[0].ant_name          # e.g. "DMASW3_49"
                lane = int(name[5:name.index("_")])
                g.queue_num = lane % 4

    nc.compile()
    return nc


def kernel(x, edge_index, batch_ids, W1, b1, W2, b2, W3, b3, lin_W, lin_b):
    global _COMPILED, _COMPILED_KEY
    x = np.asarray(x, dtype=np.float32)
    edge_index = np.asarray(edge_index)
    batch_ids = np.asarray(batch_ids)
    W1 = np.asarray(W1, np.float32); b1 = np.asarray(b1, np.float32)
    W2 = np.asarray(W2, np.float32); b2 = np.asarray(b2, np.float32)
    W3 = np.asarray(W3, np.float32); b3 = np.asarray(b3, np.float32)
    lin_W = np.asarray(lin_W, np.float32); lin_b = np.asarray(lin_b, np.float32)

    sched = _build_schedule(edge_index)
    perm, dinv = sched["perm"], sched["dinv"]

    key = (sched["Dtot_t"].tobytes(), tuple(sched["ops"]))
    if _COMPILED is None or _COMPILED_KEY != key:
        nc = _build_program(sched["Dtot_t"], sched["ops"],
                            sched["idx_cols16"])
        _COMPILED = _Runner(nc, NCORES)
        _COMPILED_KEY = key
    r = _COMPILED

    g1 = (x @ W1) * dinv[:, None]
    g1p = g1[perm]          # [100000, 30] in new node order
    dinvp = dinv[perm]

    bbc = np.stack([
        np.broadcast_to(b1, (P, HID)),
        np.broadcast_to(b2, (P, HID)),
        np.broadcast_to(b3, (P, HID)),
    ], axis=1).astype(np.float32)  # [P, 3, HID]

    idx16 = sched["idx16"]          # [NCORES, 16, idx_cols16]
    in_maps = []
    for c in range(NCORES):
        lo = c * NODES_PER_CORE
        g1own = np.zeros((SEG_PAD, ROWW), np.float32)
        g1own[1:1 + NODES_PER_CORE, :FEAT] = g1p[lo:lo + NODES_PER_CORE]
        dv = np.zeros((NTILES * P, 1), np.float32)
        dv[:NODES_PER_CORE, 0] = dinvp[lo:lo + NODES_PER_CORE]
        in_maps.append({
            "g1own": g1own,
            "idxs": np.tile(idx16[c], (8, 1)),
            "dinv": dv,
            "w2": W2, "w3": W3, "bb": bbc,
        })

    r.put_inputs(in_maps)
    res = r.call()
    results = r.results(res)

    h3p = np.concatenate(
        [results[c]["h3"][:NODES_PER_CORE] for c in range(NCORES)], axis=0)
    h3 = np.empty_like(h3p)
    h3[perm] = h3p
    pooled = np.zeros((N_GRAPHS, HID), np.float32)
    np.add.at(pooled, batch_ids.astype(np.int64), h3)
    return pooled @ lin_W + lin_b
